# revision 8
# baseline (speedup 1.0000x reference)
"""Cost-volume construction (nn_CostVolume) as a Bass/Trainium2 SPMD kernel.

Problem (hardcoded shapes):
    left_features:  (4, 32, 64, 128) f32
    right_features: (4, 32, 64, 128) f32
    max_disparity:  192  ->  D = 48
    output:         (4, 64, 48, 64, 128) f32
        out[:, :C, d]  = left
        out[:, C:, d, h, w] = right[:, :, h, w+d] if w+d < W else 0

Pure data movement (384 MiB written from 8 MiB of input) -> DMA-only kernel.
Sharding: disparity axis D=48 split 6-per-core across 8 cores.

GRADED: "mega2" — like mega but with NO row padding. Right rows are
host-shifted by d0=6k and zero-extended to 8197; window ld bleeds across
row boundaries, landing junk exactly where the output must be zero
(w + d >= W), which the host gather zeroes. Writes are output-exact
(48 MiB/core vs mega's 49.9), reads 8.0 vs 8.3 MiB. Measured ~10%
faster than mega (142-164 us/iter, env-dependent; mega 158-174).
Also tried: pshard/wide (read-dedup via image sharding — killed by 4 KiB
descriptor runs resp. 16-partition port limits), repl/repl2 (read-dedup
via on-chip 16->128 tree replication, port-disjoint + double-buffered —
exactly correct but ties mega2: the SBUF->SBUF tree throttles the ring),
mega3 (mega2 + A/B double buffering — 22% SLOWER: overlapping next-iter
loads with stores interleaves HBM reads into the sequential write
streams and costs more in row-buffer locality than the hidden load
latency is worth; serialized load->store phases win on this HBM),
mega5 (all four DMAs on one ring so the two stores serialize per-engine
into a single write stream — exact dead tie with mega2, 153.2 vs 153.4
us/iter: concurrent write streams to different regions are free; only
read-into-write interleave costs).

Key tricks:
- Right half: rows are zero-padded from W=128 to PW=133 and flattened per
  (b,c) partition, so the shifted slab for local disparity ld is exactly
  rext_flat[ld : ld + H*PW]: the shift runs across row boundaries into the
  zero padding, which provides the w+d >= W zero fill; the junk in the
  padding columns is stripped on the host. Every store is one fully
  contiguous ~4.3 MiB DMA.
- Outputs are partition-major ([p, ld, slab]) so every store's DRAM AP is
  2-dim; a channel-major layout gives 3-dim DRAM APs whose descriptors the
  DGE cannot spread across the 16 SDMA engines (measured ~3.4x slower).
- Left and right halves are fully independent chains on the two HWDGE
  rings (SP and ACT), each with its own semaphore, so the two loads and
  the 12 stores overlap.
"""

import numpy as np

import concourse.bass as bass
from concourse import mybir
from concourse.bass_utils import run_bass_kernel_spmd

B, C, H, W = 4, 32, 64, 128
D = 48
NCORES = 8
DLOC = D // NCORES          # 6 disparities per core
PW = W + DLOC - 1           # 133: padded row width (right half)
HW = H * W                  # 8192: left-half slab
SLAB = H * PW               # 8512: right-half slab
SRCW = SLAB + DLOC - 1      # right input per-partition width

_NC_CACHE = {}

# partition-shard variant ("pshard"): core k owns 16 of the 128 (b,c)
# images for BOTH halves and ALL 48 disparities. Partitions are
# (bc_local, h-group) = 16*8 = 128, each holding 8 rows (1024 f32).
# Reads drop 8x (each input element loaded exactly once chip-wide) and
# the right half needs NO row padding: window d of a group bleeds into
# the following row, landing junk exactly where the output must be zero
# (w + d >= W), which the host gather zeroes. Writes are output-exact.
PBC = (B * C) // NCORES     # 16 (b,c) images per core
PG = 8                      # h-groups per image
GR = H // PG                # 8 rows per group
GW = GR * W                 # 1024: flat row-group width
PSRC = GW + D - 1           # 1071: group + 47-element spill tail

# wide variant: like pshard but one WHOLE image per partition (16
# partitions x 8192 f32 = 32 KiB contiguous chunks, the size at which the
# SDMA engines hit full rate; pshard's 4 KiB chunks were descriptor-
# overhead-bound). Window d of an image bleeds across row boundaries,
# landing junk exactly in the w+d >= W region the gather zeroes.
# Measured 5x WORSE than mega: 16 partitions engage only 4 of the 16
# SBUF AXI ports (8 partitions per port) -> ~110 GB/s ceiling.
WSRC = HW + D - 1           # 8239: image + 47-element zero tail

# mega2: disparity-shard like mega (128 partitions, 32 KiB runs) but
# without the 133-wide row padding: windows bleed across rows and the
# host zeroes the w+d >= W junk. Input rows are host-shifted by d0=6k.
M2W = HW + DLOC - 1         # 8197: shifted image + window tail

# channel-shard variant ("cshard"): core k=(ch,dq) handles channels
# 16ch..16ch+16 of both halves and disparities 12dq..12dq+12; partitions are
# (b, c, h-half) = 4*16*2 = 128. Reads drop to ~half of each input per core.
CH = C // 2                 # 16 channels per core
NH = 2                      # h-groups per partition split
HR = H // NH                # 32 rows per group
DL5 = D // 4                # 12 disparities per core
PW5 = W + DL5 - 1           # 139 padded row width
LSLAB = HR * W              # 4096
RSLAB = HR * PW5            # 4448
RSRC = RSLAB + DL5 - 1      # 4459


def _build(repeat=1, variant="mega"):
    """Build the SPMD program. repeat>1 re-runs the whole body that many
    times, serialized per chain on its semaphore — used only for
    steady-state benchmarking; the graded path uses repeat=1.
    variant: "mega" = one store per half; "multi" = one store per ld."""
    if (repeat, variant) in _NC_CACHE:
        return _NC_CACHE[(repeat, variant)]
    if variant == "cshard":
        nc = _build_windows(repeat, DL5, LSLAB, RSLAB, RSRC)
        _NC_CACHE[(repeat, variant)] = nc
        return nc
    if variant == "pshard":
        nc = _build_windows(repeat, D, GW, GW, PSRC)
        _NC_CACHE[(repeat, variant)] = nc
        return nc
    if variant == "wide":
        nc = _build_windows(repeat, D, HW, HW, WSRC, nparts=PBC)
        _NC_CACHE[(repeat, variant)] = nc
        return nc
    if variant == "mega2":
        nc = _build_windows(repeat, DLOC, HW, HW, M2W)
        _NC_CACHE[(repeat, variant)] = nc
        return nc
    if variant == "wind":
        nc = _build_wind(repeat)
        _NC_CACHE[(repeat, variant)] = nc
        return nc
    if variant == "wind2":
        nc = _build_wind2(repeat)
        _NC_CACHE[(repeat, variant)] = nc
        return nc
    if variant == "wind4":
        nc = _build_wind4(repeat)
        _NC_CACHE[(repeat, variant)] = nc
        return nc
    if variant == "repl":
        nc = _build_repl(repeat)
        _NC_CACHE[(repeat, variant)] = nc
        return nc
    if variant == "repl2":
        nc = _build_repl2(repeat)
        _NC_CACHE[(repeat, variant)] = nc
        return nc
    if variant == "mega3":
        nc = _build_mega3(repeat)
        _NC_CACHE[(repeat, variant)] = nc
        return nc
    if variant == "mega5":
        nc = _build_mega5(repeat)
        _NC_CACHE[(repeat, variant)] = nc
        return nc
    if variant == "hpipe":
        nc = _build_hpipe(repeat)
        _NC_CACHE[(repeat, variant)] = nc
        return nc
    nc = bass.Bass()
    left_in = nc.declare_dram_parameter(
        "left", [B * C, HW], mybir.dt.float32, isOutput=False)
    rext_in = nc.declare_dram_parameter(
        "rext", [B * C, SRCW], mybir.dt.float32, isOutput=False)
    out_l = nc.declare_dram_parameter(
        "out_l", [B * C, DLOC, HW], mybir.dt.float32, isOutput=True)
    out_r = nc.declare_dram_parameter(
        "out_r", [B * C, DLOC, SLAB], mybir.dt.float32, isOutput=True)

    if variant == "pipe":
        nc = _build_pipe(nc, repeat, left_in, rext_in, out_l, out_r)
        _NC_CACHE[(repeat, variant)] = nc
        return nc

    with (
        nc.sbuf_tensor([B * C, HW], mybir.dt.float32) as left_t,
        nc.sbuf_tensor([B * C, SRCW], mybir.dt.float32) as rext_t,
        nc.semaphore("sem_l") as sem_l,
        nc.semaphore("sem_r") as sem_r,
        nc.Block() as block,
    ):
        n_store = 1 if variant == "mega" else DLOC
        per_iter = (1 + n_store) * 16

        @block.sync
        def _(sync):
            for r in range(repeat):
                base = per_iter * r
                if r:
                    sync.wait_ge(sem_l, base)
                sync.dma_start(left_t[:], left_in[:]).then_inc(sem_l, 16)
                sync.wait_ge(sem_l, base + 16)
                if variant == "mega":
                    # one store: broadcast left over DLOC via a step-0 dim
                    bsrc = left_t[:, None, :].broadcast_to((B * C, DLOC, HW))
                    sync.dma_start(out_l[:, :, :], bsrc).then_inc(sem_l, 16)
                else:
                    for ld in range(DLOC):
                        sync.dma_start(
                            out_l[:, ld, :], left_t[:]
                        ).then_inc(sem_l, 16)
            sync.wait_ge(sem_l, per_iter * repeat)

        @block.scalar
        def _(scalar):
            for r in range(repeat):
                base = per_iter * r
                if r:
                    scalar.wait_ge(sem_r, base)
                scalar.dma_start(rext_t[:], rext_in[:]).then_inc(sem_r, 16)
                scalar.wait_ge(sem_r, base + 16)
                if variant == "mega":
                    # one store: DLOC overlapping shifted windows
                    rbase = rext_t[:]
                    wsrc = type(rbase)(
                        rbase.tensor, rbase.offset,
                        [list(rbase.ap[0]), [1, DLOC], [1, SLAB]],
                    )
                    scalar.dma_start(out_r[:, :, :], wsrc).then_inc(sem_r, 16)
                else:
                    for ld in range(DLOC):
                        scalar.dma_start(
                            out_r[:, ld, :], rext_t[:, ld:ld + SLAB]
                        ).then_inc(sem_r, 16)
            scalar.wait_ge(sem_r, per_iter * repeat)

    _NC_CACHE[(repeat, variant)] = nc
    return nc


def _build_windows(repeat, nwin, lslab, rslab, rsrc, nparts=128):
    """Generic two-chain mega-store program: left broadcast over nwin
    windows of lslab; right = nwin shifted windows of rslab from a padded
    flat source of width rsrc."""
    nc = bass.Bass()
    left_in = nc.declare_dram_parameter(
        "left", [nparts, lslab], mybir.dt.float32, isOutput=False)
    rext_in = nc.declare_dram_parameter(
        "rext", [nparts, rsrc], mybir.dt.float32, isOutput=False)
    out_l = nc.declare_dram_parameter(
        "out_l", [nparts, nwin, lslab], mybir.dt.float32, isOutput=True)
    out_r = nc.declare_dram_parameter(
        "out_r", [nparts, nwin, rslab], mybir.dt.float32, isOutput=True)

    with (
        nc.sbuf_tensor([nparts, lslab], mybir.dt.float32) as left_t,
        nc.sbuf_tensor([nparts, rsrc], mybir.dt.float32) as rext_t,
        nc.semaphore("sem_l") as sem_l,
        nc.semaphore("sem_r") as sem_r,
        nc.Block() as block,
    ):
        per_iter = 2 * 16

        @block.sync
        def _(sync):
            for r in range(repeat):
                base = per_iter * r
                if r:
                    sync.wait_ge(sem_l, base)
                sync.dma_start(left_t[:], left_in[:]).then_inc(sem_l, 16)
                sync.wait_ge(sem_l, base + 16)
                bsrc = left_t[:, None, :].broadcast_to((nparts, nwin, lslab))
                sync.dma_start(out_l[:, :, :], bsrc).then_inc(sem_l, 16)
            sync.wait_ge(sem_l, per_iter * repeat)

        @block.scalar
        def _(scalar):
            for r in range(repeat):
                base = per_iter * r
                if r:
                    scalar.wait_ge(sem_r, base)
                scalar.dma_start(rext_t[:], rext_in[:]).then_inc(sem_r, 16)
                scalar.wait_ge(sem_r, base + 16)
                rbase = rext_t[:]
                wsrc = type(rbase)(
                    rbase.tensor, rbase.offset,
                    [list(rbase.ap[0]), [1, nwin], [1, rslab]],
                )
                scalar.dma_start(out_r[:, :, :], wsrc).then_inc(sem_r, 16)
            scalar.wait_ge(sem_r, per_iter * repeat)

    return nc


def _host_inputs_cshard(left, right):
    """Per-core inputs for the channel-shard variant. Core k = ch*4 + dq:
    channels [16ch, 16ch+16), disparities [12dq, 12dq+12). Partition
    p = b*32 + c*2 + hh holds rows [32hh, 32hh+32)."""
    in_maps = []
    # (b, c16, hh, HR, W) view per half-channel group
    lv = left.reshape(B, 2, CH, H, W)   # c = 16ch + c16 -> split (2, 16)
    rv = right.reshape(B, 2, CH, H, W)
    for k in range(NCORES):
        ch, dq = divmod(k, 4)
        d0 = DL5 * dq
        lslice = lv[:, ch]              # (B, CH, H, W)
        rslice = rv[:, ch]
        # partitions (b, c, hh)
        lp = lslice.reshape(B, CH, NH, HR, W).transpose(0, 1, 2, 3, 4)
        le = np.ascontiguousarray(lp).reshape(B * CH * NH, LSLAB)
        re = np.zeros((B, CH, NH, HR, PW5), np.float32)
        take = max(0, W - d0)
        re[:, :, :, :, :take] = rslice.reshape(B, CH, NH, HR, W)[:, :, :, :, d0:d0 + take]
        re_flat = np.zeros((B * CH * NH, RSRC), np.float32)
        re_flat[:, :RSLAB] = re.reshape(B * CH * NH, RSLAB)
        # group-boundary spill: windows read up to DL5-1 elements past the
        # group's flat end; for hh=0 that region is the head of row 32
        # (start of hh=1's group), for hh=1 it is past the image (zeros,
        # but those reads only land in stripped padding columns anyway).
        spill = re.reshape(B, CH, NH, RSLAB)
        re3 = re_flat.reshape(B, CH, NH, RSRC)
        re3[:, :, 0, RSLAB:] = spill[:, :, 1, :DL5 - 1]
        in_maps.append({"left": le, "rext": re_flat})
    return in_maps


def _gather_cshard(results):
    out = np.empty((B, 2 * C, D, H, W), np.float32)
    for k in range(NCORES):
        ch, dq = divmod(k, 4)
        csl = slice(CH * ch, CH * (ch + 1))
        dsl = slice(DL5 * dq, DL5 * (dq + 1))
        ol = results[k]["out_l"].reshape(B, CH, NH, DL5, HR, W)
        out[:, csl, dsl] = ol.transpose(0, 1, 3, 2, 4, 5).reshape(
            B, CH, DL5, H, W)
        orr = results[k]["out_r"].reshape(B, CH, NH, DL5, HR, PW5)
        out[:, C + CH * ch:C + CH * (ch + 1), dsl] = (
            orr[:, :, :, :, :, :W].transpose(0, 1, 3, 2, 4, 5).reshape(
                B, CH, DL5, H, W)
        )
    return out


def _host_inputs_pshard(left, right):
    """Per-core inputs for the partition-shard variant. Core k owns
    (b,c) images [16k, 16k+16); partition p = bc_local*8 + g holds rows
    [8g, 8g+8) flattened to 1024 f32. The right buffer appends a 47-zero
    spill tail — every element a window reads from it lands in an output
    position the gather zeroes anyway."""
    lv = left.reshape(B * C, H, W)
    rv = right.reshape(B * C, H, W)
    in_maps = []
    for k in range(NCORES):
        sl = lv[PBC * k:PBC * (k + 1)]
        le = np.ascontiguousarray(sl).reshape(PBC * PG, GW)
        re = np.zeros((PBC * PG, PSRC), np.float32)
        re[:, :GW] = rv[PBC * k:PBC * (k + 1)].reshape(PBC * PG, GW)
        in_maps.append({"left": le, "rext": re})
    return in_maps


def _gather_pshard(results):
    out = np.empty((B, 2 * C, D, H, W), np.float32)
    for k in range(NCORES):
        # PBC=16 divides C=32, so core k's bc range is one b, contiguous c
        b0, c0 = divmod(PBC * k, C)
        ol = results[k]["out_l"].reshape(PBC, PG, D, GR, W)
        out[b0, c0:c0 + PBC] = ol.transpose(0, 2, 1, 3, 4).reshape(PBC, D, H, W)
        orr = results[k]["out_r"].reshape(PBC, PG, D, GR, W)
        out[b0, C + c0:C + c0 + PBC] = orr.transpose(0, 2, 1, 3, 4).reshape(
            PBC, D, H, W)
    # zero the junk the row-bleeding windows wrote: exactly w + d >= W
    rh = out[:, C:]
    for d in range(1, D):
        rh[:, :, d, :, W - d:] = 0
    return out


RW = HW + D - 1             # 8239: repl right input width (47-zero tail)


def _build_wind(repeat):
    """Read-deduplicated image shard with NO replication: core k owns 16
    (b,c) images of both halves and writes all 48 disparities for them.
    Loads drop 8x chip-wide (1.05 MiB/core vs mega2's 8.4) and there is
    no SBUF->SBUF tree: the stores read the 16 source partitions through
    window APs directly — left broadcast over 48 step-0 windows, right as
    48 step-1 shifted windows of the 8239-wide padded source. The 16
    partitions are spread at stride 8 (left at p=8i, right at p=8i+4) so
    every SBUF AXI port serves exactly one left and one right partition
    (16 consecutive partitions engaged too few ports — the "wide"
    failure). Row-bleed junk lands exactly at w+d >= W, host-zeroed."""
    nc = bass.Bass()
    left_in = nc.declare_dram_parameter(
        "left", [PBC, HW], mybir.dt.float32, isOutput=False)
    rext_in = nc.declare_dram_parameter(
        "rext", [PBC, WSRC], mybir.dt.float32, isOutput=False)
    out_l = nc.declare_dram_parameter(
        "out_l", [PBC, D, HW], mybir.dt.float32, isOutput=True)
    out_r = nc.declare_dram_parameter(
        "out_r", [PBC, D, HW], mybir.dt.float32, isOutput=True)

    with (
        nc.sbuf_tensor([128, HW], mybir.dt.float32) as left_t,
        nc.sbuf_tensor([128, WSRC], mybir.dt.float32) as rext_t,
        nc.semaphore("sem_l") as sem_l,
        nc.semaphore("sem_r") as sem_r,
        nc.Block() as block,
    ):
        per_iter = 2 * 16

        @block.sync
        def _(sync):
            for r in range(repeat):
                base = per_iter * r
                if r:
                    sync.wait_ge(sem_l, base)
                sync.dma_start(left_t[0::8, :], left_in[:]).then_inc(sem_l, 16)
                sync.wait_ge(sem_l, base + 16)
                bsrc = left_t[0::8, None, :].broadcast_to((PBC, D, HW))
                sync.dma_start(out_l[:, :, :], bsrc).then_inc(sem_l, 16)
            sync.wait_ge(sem_l, per_iter * repeat)

        @block.scalar
        def _(scalar):
            for r in range(repeat):
                base = per_iter * r
                if r:
                    scalar.wait_ge(sem_r, base)
                scalar.dma_start(
                    rext_t[4::8, :], rext_in[:]).then_inc(sem_r, 16)
                scalar.wait_ge(sem_r, base + 16)
                rb = rext_t[4::8, :]
                wsrc = type(rb)(
                    rb.tensor, rb.offset,
                    [list(rb.ap[0]), [1, D], [1, HW]],
                )
                scalar.dma_start(out_r[:, :, :], wsrc).then_inc(sem_r, 16)
            scalar.wait_ge(sem_r, per_iter * repeat)

    return nc


def _ap(base, offset, ap):
    """Raw AP constructor (same trick the mega variants use)."""
    return type(base)(base.tensor, offset, ap)


def _build_wind2(repeat):
    """wind + ONE replication level, copies interleaved in partition
    order so each store is a single 3-dim AP over 32 partitions.
    Left: image i copy c at p = 8i+4c (identical copies). Right: at
    p = 8i+4c+2, copy 1 pre-shifted by 24. Disparity d = 24c + u; the
    DRAM row order (i, c) matches the output layout. Tree cost: 1.05 MiB
    SBUF->SBUF total (vs repl's 14)."""
    nc = bass.Bass()
    left_in = nc.declare_dram_parameter(
        "left", [PBC, HW], mybir.dt.float32, isOutput=False)
    rext_in = nc.declare_dram_parameter(
        "rext", [PBC, WSRC], mybir.dt.float32, isOutput=False)
    out_l = nc.declare_dram_parameter(
        "out_l", [PBC * 2, 24, HW], mybir.dt.float32, isOutput=True)
    out_r = nc.declare_dram_parameter(
        "out_r", [PBC * 2, 24, HW], mybir.dt.float32, isOutput=True)

    with (
        nc.sbuf_tensor([128, HW], mybir.dt.float32) as left_t,
        nc.sbuf_tensor([128, WSRC], mybir.dt.float32) as rext_t,
        nc.semaphore("sem_l") as sem_l,
        nc.semaphore("sem_r") as sem_r,
        nc.Block() as block,
    ):
        per_iter = 3 * 16

        @block.sync
        def _(sync):
            for r in range(repeat):
                base = per_iter * r
                if r:
                    sync.wait_ge(sem_l, base)
                sync.dma_start(left_t[0::8, :], left_in[:]).then_inc(sem_l, 16)
                sync.wait_ge(sem_l, base + 16)
                sync.dma_start(
                    left_t[4::8, :], left_t[0::8, :]).then_inc(sem_l, 16)
                sync.wait_ge(sem_l, base + 32)
                lb = left_t[0::4, :]
                src = _ap(lb, lb.offset,
                          [list(lb.ap[0]), [0, 24], [1, HW]])
                sync.dma_start(out_l[:, :, :], src).then_inc(sem_l, 16)
            sync.wait_ge(sem_l, per_iter * repeat)

        @block.scalar
        def _(scalar):
            for r in range(repeat):
                base = per_iter * r
                if r:
                    scalar.wait_ge(sem_r, base)
                scalar.dma_start(
                    rext_t[2::8, :], rext_in[:]).then_inc(sem_r, 16)
                scalar.wait_ge(sem_r, base + 16)
                # copy 1 = copy 0 shifted 24; windows u<=23 read cols
                # <= 23+8191 = 8214 < WSRC-24 = 8215.
                scalar.dma_start(
                    rext_t[6::8, 0:WSRC - 24],
                    rext_t[2::8, 24:WSRC]).then_inc(sem_r, 16)
                scalar.wait_ge(sem_r, base + 32)
                rb = rext_t[2::4, :]
                src = _ap(rb, rb.offset,
                          [list(rb.ap[0]), [1, 24], [1, HW]])
                scalar.dma_start(out_r[:, :, :], src).then_inc(sem_r, 16)
            scalar.wait_ge(sem_r, per_iter * repeat)

    return nc


def _build_wind4(repeat):
    """wind + TWO replication levels: all 128 partitions active, copies
    interleaved so each store is one 3-dim AP over 64 partitions. Left:
    image i copy c at p = 8i+2c (identical). Right: at p = 8i+2c+1 with
    pre-shift 12c baked in by the tree. Disparity d = 12c + u. Tree:
    3.1 MiB SBUF->SBUF total."""
    nc = bass.Bass()
    left_in = nc.declare_dram_parameter(
        "left", [PBC, HW], mybir.dt.float32, isOutput=False)
    rext_in = nc.declare_dram_parameter(
        "rext", [PBC, WSRC], mybir.dt.float32, isOutput=False)
    out_l = nc.declare_dram_parameter(
        "out_l", [PBC * 4, 12, HW], mybir.dt.float32, isOutput=True)
    out_r = nc.declare_dram_parameter(
        "out_r", [PBC * 4, 12, HW], mybir.dt.float32, isOutput=True)

    with (
        nc.sbuf_tensor([128, HW], mybir.dt.float32) as left_t,
        nc.sbuf_tensor([128, WSRC], mybir.dt.float32) as rext_t,
        nc.semaphore("sem_l") as sem_l,
        nc.semaphore("sem_r") as sem_r,
        nc.Block() as block,
    ):
        per_iter = 4 * 16

        @block.sync
        def _(sync):
            for r in range(repeat):
                base = per_iter * r
                if r:
                    sync.wait_ge(sem_l, base)
                sync.dma_start(left_t[0::8, :], left_in[:]).then_inc(sem_l, 16)
                sync.wait_ge(sem_l, base + 16)
                sync.dma_start(
                    left_t[4::8, :], left_t[0::8, :]).then_inc(sem_l, 16)
                sync.wait_ge(sem_l, base + 32)
                sync.dma_start(
                    left_t[2::4, :], left_t[0::4, :]).then_inc(sem_l, 16)
                sync.wait_ge(sem_l, base + 48)
                lb = left_t[0::2, :]
                src = _ap(lb, lb.offset,
                          [list(lb.ap[0]), [0, 12], [1, HW]])
                sync.dma_start(out_l[:, :, :], src).then_inc(sem_l, 16)
            sync.wait_ge(sem_l, per_iter * repeat)

        @block.scalar
        def _(scalar):
            for r in range(repeat):
                base = per_iter * r
                if r:
                    scalar.wait_ge(sem_r, base)
                scalar.dma_start(
                    rext_t[1::8, :], rext_in[:]).then_inc(sem_r, 16)
                scalar.wait_ge(sem_r, base + 16)
                # c=2 (p=8i+5) = c=0 shifted 24.
                scalar.dma_start(
                    rext_t[5::8, 0:WSRC - 24],
                    rext_t[1::8, 24:WSRC]).then_inc(sem_r, 16)
                scalar.wait_ge(sem_r, base + 32)
                # {c=1, c=3} = {c=0, c=2} shifted 12. Width WSRC-36: the
                # shift-36 copy reads orig up to 12+8202 = WSRC-25+... and
                # store windows u<=11 read copies up to col 11+8191 = 8202
                # = WSRC-37, inside the written [0, WSRC-36) range.
                scalar.dma_start(
                    rext_t[3::4, 0:WSRC - 36],
                    rext_t[1::4, 12:WSRC - 24]).then_inc(sem_r, 16)
                scalar.wait_ge(sem_r, base + 48)
                rb = rext_t[1::2, :]
                src = _ap(rb, rb.offset,
                          [list(rb.ap[0]), [1, 12], [1, HW]])
                scalar.dma_start(out_r[:, :, :], src).then_inc(sem_r, 16)
            scalar.wait_ge(sem_r, per_iter * repeat)

    return nc


def _build_repl(repeat):
    """Read-deduplicated disparity shard. Core k loads only its 16 images
    per half (1.03 MiB total HBM reads, 8x less than mega), tree-
    replicates them 16->32->64->128 partitions via SBUF->SBUF DMA (no HBM
    traffic), each copy pre-shifted by 6g elements so partition p=g*16+i
    holds image i shifted by 6g and stores disparities [6g, 6g+6) with a
    uniform window AP. Writes are output-exact (48 MiB); junk from
    row-bleed windows lands in the w+d >= W region the host zeroes."""
    nc = bass.Bass()
    left_in = nc.declare_dram_parameter(
        "left", [PBC, HW], mybir.dt.float32, isOutput=False)
    rext_in = nc.declare_dram_parameter(
        "rext", [PBC, RW], mybir.dt.float32, isOutput=False)
    out_l = nc.declare_dram_parameter(
        "out_l", [128, DLOC, HW], mybir.dt.float32, isOutput=True)
    out_r = nc.declare_dram_parameter(
        "out_r", [128, DLOC, HW], mybir.dt.float32, isOutput=True)

    with (
        nc.sbuf_tensor([128, HW], mybir.dt.float32) as left_t,
        nc.sbuf_tensor([128, RW], mybir.dt.float32) as rext_t,
        nc.semaphore("sem_l") as sem_l,
        nc.semaphore("sem_r") as sem_r,
        nc.Block() as block,
    ):
        per_iter = 5 * 16   # load + 3 tree copies + store

        @block.sync
        def _(sync):
            for r in range(repeat):
                base = per_iter * r
                if r:
                    sync.wait_ge(sem_l, base)
                sync.dma_start(left_t[0:16, :], left_in[:]).then_inc(sem_l, 16)
                sync.wait_ge(sem_l, base + 16)
                sync.dma_start(
                    left_t[16:32, :], left_t[0:16, :]).then_inc(sem_l, 16)
                sync.wait_ge(sem_l, base + 32)
                sync.dma_start(
                    left_t[32:64, :], left_t[0:32, :]).then_inc(sem_l, 16)
                sync.wait_ge(sem_l, base + 48)
                sync.dma_start(
                    left_t[64:128, :], left_t[0:64, :]).then_inc(sem_l, 16)
                sync.wait_ge(sem_l, base + 64)
                bsrc = left_t[:, None, :].broadcast_to((128, DLOC, HW))
                sync.dma_start(out_l[:, :, :], bsrc).then_inc(sem_l, 16)
            sync.wait_ge(sem_l, per_iter * repeat)

        @block.scalar
        def _(scalar):
            for r in range(repeat):
                base = per_iter * r
                if r:
                    scalar.wait_ge(sem_r, base)
                scalar.dma_start(
                    rext_t[0:16, :], rext_in[:]).then_inc(sem_r, 16)
                scalar.wait_ge(sem_r, base + 16)
                # tree copies shift by 6 images' disparities per doubling:
                # partition 16g+i ends up holding image i shifted by 6g
                scalar.dma_start(
                    rext_t[16:32, 0:RW - 6],
                    rext_t[0:16, 6:RW]).then_inc(sem_r, 16)
                scalar.wait_ge(sem_r, base + 32)
                scalar.dma_start(
                    rext_t[32:64, 0:RW - 12],
                    rext_t[0:32, 12:RW]).then_inc(sem_r, 16)
                scalar.wait_ge(sem_r, base + 48)
                scalar.dma_start(
                    rext_t[64:128, 0:RW - 24],
                    rext_t[0:64, 24:RW]).then_inc(sem_r, 16)
                scalar.wait_ge(sem_r, base + 64)
                rbase = rext_t[:]
                wsrc = type(rbase)(
                    rbase.tensor, rbase.offset,
                    [list(rbase.ap[0]), [1, DLOC], [1, HW]],
                )
                scalar.dma_start(out_r[:, :, :], wsrc).then_inc(sem_r, 16)
            scalar.wait_ge(sem_r, per_iter * repeat)

    return nc


def _build_mega3(repeat):
    """mega2 + A/B double buffering: iteration r+1's 4 MiB loads overlap
    iteration r's stores instead of serializing after them, removing the
    load latency from the steady-state critical path."""
    nc = bass.Bass()
    left_in = nc.declare_dram_parameter(
        "left", [128, HW], mybir.dt.float32, isOutput=False)
    rext_in = nc.declare_dram_parameter(
        "rext", [128, M2W], mybir.dt.float32, isOutput=False)
    out_l = nc.declare_dram_parameter(
        "out_l", [128, DLOC, HW], mybir.dt.float32, isOutput=True)
    out_r = nc.declare_dram_parameter(
        "out_r", [128, DLOC, HW], mybir.dt.float32, isOutput=True)

    with (
        nc.sbuf_tensor([128, HW], mybir.dt.float32) as left_a,
        nc.sbuf_tensor([128, HW], mybir.dt.float32) as left_b,
        nc.sbuf_tensor([128, M2W], mybir.dt.float32) as rext_a,
        nc.sbuf_tensor([128, M2W], mybir.dt.float32) as rext_b,
        nc.semaphore("sem_l") as sem_l,
        nc.semaphore("sem_r") as sem_r,
        nc.Block() as block,
    ):
        per_iter = 2 * 16

        @block.sync
        def _(sync):
            for r in range(repeat):
                base = per_iter * r
                lt = left_a if r % 2 == 0 else left_b
                if r >= 2:
                    sync.wait_ge(sem_l, per_iter * (r - 1))
                sync.dma_start(lt[:], left_in[:]).then_inc(sem_l, 16)
                sync.wait_ge(sem_l, base + 16)
                bsrc = lt[:, None, :].broadcast_to((128, DLOC, HW))
                sync.dma_start(out_l[:, :, :], bsrc).then_inc(sem_l, 16)
            sync.wait_ge(sem_l, per_iter * repeat)

        @block.scalar
        def _(scalar):
            for r in range(repeat):
                base = per_iter * r
                rt = rext_a if r % 2 == 0 else rext_b
                if r >= 2:
                    scalar.wait_ge(sem_r, per_iter * (r - 1))
                scalar.dma_start(rt[:], rext_in[:]).then_inc(sem_r, 16)
                scalar.wait_ge(sem_r, base + 16)
                rbase = rt[:]
                wsrc = type(rbase)(
                    rbase.tensor, rbase.offset,
                    [list(rbase.ap[0]), [1, DLOC], [1, HW]],
                )
                scalar.dma_start(out_r[:, :, :], wsrc).then_inc(sem_r, 16)
            scalar.wait_ge(sem_r, per_iter * repeat)

    return nc


def _build_mega5(repeat):
    """mega2 with all four DMAs on ONE ring: per-engine FIFO serializes
    the two stores, so HBM sees a single sequential write stream at a
    time instead of two interleaved ones."""
    nc = bass.Bass()
    left_in = nc.declare_dram_parameter(
        "left", [128, HW], mybir.dt.float32, isOutput=False)
    rext_in = nc.declare_dram_parameter(
        "rext", [128, M2W], mybir.dt.float32, isOutput=False)
    out_l = nc.declare_dram_parameter(
        "out_l", [128, DLOC, HW], mybir.dt.float32, isOutput=True)
    out_r = nc.declare_dram_parameter(
        "out_r", [128, DLOC, HW], mybir.dt.float32, isOutput=True)

    with (
        nc.sbuf_tensor([128, HW], mybir.dt.float32) as left_t,
        nc.sbuf_tensor([128, M2W], mybir.dt.float32) as rext_t,
        nc.semaphore("sem") as sem,
        nc.Block() as block,
    ):
        per_iter = 4 * 16

        @block.sync
        def _(sync):
            for r in range(repeat):
                base = per_iter * r
                if r:
                    sync.wait_ge(sem, base)
                sync.dma_start(left_t[:], left_in[:]).then_inc(sem, 16)
                sync.dma_start(rext_t[:], rext_in[:]).then_inc(sem, 16)
                sync.wait_ge(sem, base + 32)
                bsrc = left_t[:, None, :].broadcast_to((128, DLOC, HW))
                sync.dma_start(out_l[:, :, :], bsrc).then_inc(sem, 16)
                rbase = rext_t[:]
                wsrc = type(rbase)(
                    rbase.tensor, rbase.offset,
                    [list(rbase.ap[0]), [1, DLOC], [1, HW]],
                )
                sync.dma_start(out_r[:, :, :], wsrc).then_inc(sem, 16)
            sync.wait_ge(sem, per_iter * repeat)

    return nc


def _build_repl2(repeat):
    """repl + port-disjoint trees + double buffering. Left tree is rooted
    at partitions [0:16) (even SBUF ports), right tree at [64:80) (odd
    ports), so the two chains' loads and copies never share AXI ports.
    A/B buffers let iteration r+1's load+tree overlap iteration r's
    stores. Right block b=p//16 holds shift 6*g with g=(b+4)%8."""
    nc = bass.Bass()
    left_in = nc.declare_dram_parameter(
        "left", [PBC, HW], mybir.dt.float32, isOutput=False)
    rext_in = nc.declare_dram_parameter(
        "rext", [PBC, RW], mybir.dt.float32, isOutput=False)
    out_l = nc.declare_dram_parameter(
        "out_l", [128, DLOC, HW], mybir.dt.float32, isOutput=True)
    out_r = nc.declare_dram_parameter(
        "out_r", [128, DLOC, HW], mybir.dt.float32, isOutput=True)

    with (
        nc.sbuf_tensor([128, HW], mybir.dt.float32) as left_a,
        nc.sbuf_tensor([128, HW], mybir.dt.float32) as left_b,
        nc.sbuf_tensor([128, RW], mybir.dt.float32) as rext_a,
        nc.sbuf_tensor([128, RW], mybir.dt.float32) as rext_b,
        nc.semaphore("sem_l") as sem_l,
        nc.semaphore("sem_r") as sem_r,
        nc.Block() as block,
    ):
        per_iter = 5 * 16   # load + 3 tree copies + store

        @block.sync
        def _(sync):
            for r in range(repeat):
                base = per_iter * r
                lt = left_a if r % 2 == 0 else left_b
                if r >= 2:
                    sync.wait_ge(sem_l, per_iter * (r - 1))
                sync.dma_start(lt[0:16, :], left_in[:]).then_inc(sem_l, 16)
                sync.wait_ge(sem_l, base + 16)
                sync.dma_start(lt[16:32, :], lt[0:16, :]).then_inc(sem_l, 16)
                sync.wait_ge(sem_l, base + 32)
                sync.dma_start(lt[32:64, :], lt[0:32, :]).then_inc(sem_l, 16)
                sync.wait_ge(sem_l, base + 48)
                sync.dma_start(lt[64:128, :], lt[0:64, :]).then_inc(sem_l, 16)
                sync.wait_ge(sem_l, base + 64)
                bsrc = lt[:, None, :].broadcast_to((128, DLOC, HW))
                sync.dma_start(out_l[:, :, :], bsrc).then_inc(sem_l, 16)
            sync.wait_ge(sem_l, per_iter * repeat)

        @block.scalar
        def _(scalar):
            for r in range(repeat):
                base = per_iter * r
                rt = rext_a if r % 2 == 0 else rext_b
                if r >= 2:
                    scalar.wait_ge(sem_r, per_iter * (r - 1))
                scalar.dma_start(rt[64:80, :], rext_in[:]).then_inc(sem_r, 16)
                scalar.wait_ge(sem_r, base + 16)
                scalar.dma_start(
                    rt[80:96, 0:RW - 6], rt[64:80, 6:RW]).then_inc(sem_r, 16)
                scalar.wait_ge(sem_r, base + 32)
                scalar.dma_start(
                    rt[96:128, 0:RW - 12],
                    rt[64:96, 12:RW]).then_inc(sem_r, 16)
                scalar.wait_ge(sem_r, base + 48)
                scalar.dma_start(
                    rt[0:64, 0:RW - 24], rt[64:128, 24:RW]).then_inc(sem_r, 16)
                scalar.wait_ge(sem_r, base + 64)
                rbase = rt[:]
                wsrc = type(rbase)(
                    rbase.tensor, rbase.offset,
                    [list(rbase.ap[0]), [1, DLOC], [1, HW]],
                )
                scalar.dma_start(out_r[:, :, :], wsrc).then_inc(sem_r, 16)
            scalar.wait_ge(sem_r, per_iter * repeat)

    return nc


def _gather_repl2(results):
    out = np.empty((B, 2 * C, D, H, W), np.float32)
    for k in range(NCORES):
        b0, c0 = divmod(PBC * k, C)
        # left: block b holds disparities [6b, 6b+6) of image i=p%16
        ol = results[k]["out_l"].reshape(NCORES, PBC, DLOC, H, W)
        out[b0, c0:c0 + PBC] = ol.transpose(1, 0, 2, 3, 4).reshape(
            PBC, D, H, W)
        # right: block b holds shift 6*g with g = (b+4) % 8
        orr = results[k]["out_r"].reshape(NCORES, PBC, DLOC, H, W)
        gorder = [(b + 4) % 8 for b in range(NCORES)]
        inv = np.argsort(gorder)
        out[b0, C + c0:C + c0 + PBC] = orr[inv].transpose(
            1, 0, 2, 3, 4).reshape(PBC, D, H, W)
    rh = out[:, C:]
    for d in range(1, D):
        rh[:, :, d, :, W - d:] = 0
    return out


def _host_inputs_repl(left, right):
    """Core k loads only images [16k, 16k+16) per half; right gets a
    47-zero tail (windows read it only into to-be-zeroed positions)."""
    lv = left.reshape(B * C, HW)
    rv = right.reshape(B * C, HW)
    in_maps = []
    for k in range(NCORES):
        sl = slice(PBC * k, PBC * (k + 1))
        re = np.zeros((PBC, RW), np.float32)
        re[:, :HW] = rv[sl]
        in_maps.append({"left": np.ascontiguousarray(lv[sl]), "rext": re})
    return in_maps


def _gather_repl(results):
    out = np.empty((B, 2 * C, D, H, W), np.float32)
    for k in range(NCORES):
        b0, c0 = divmod(PBC * k, C)
        # device row p = g*16 + i -> (image i, disparities [6g, 6g+6))
        ol = results[k]["out_l"].reshape(NCORES, PBC, DLOC, H, W)
        out[b0, c0:c0 + PBC] = ol.transpose(1, 0, 2, 3, 4).reshape(
            PBC, D, H, W)
        orr = results[k]["out_r"].reshape(NCORES, PBC, DLOC, H, W)
        out[b0, C + c0:C + c0 + PBC] = orr.transpose(1, 0, 2, 3, 4).reshape(
            PBC, D, H, W)
    rh = out[:, C:]
    for d in range(1, D):
        rh[:, :, d, :, W - d:] = 0
    return out


def _host_inputs_mega2(left, right):
    """Unpadded disparity shard: core k gets all 128 images, right rows
    pre-shifted by d0 = 6k and zero-extended to 8197."""
    lv = left.reshape(B * C, HW)
    rv = right.reshape(B * C, HW)
    le = np.ascontiguousarray(lv)
    in_maps = []
    for k in range(NCORES):
        d0 = DLOC * k
        re = np.zeros((B * C, M2W), np.float32)
        re[:, :HW - d0] = rv[:, d0:]
        in_maps.append({"left": le, "rext": re})
    return in_maps


def _gather_mega2(results):
    out = np.empty((B, 2 * C, D, H, W), np.float32)
    for k in range(NCORES):
        dsl = slice(DLOC * k, DLOC * (k + 1))
        out[:, :C, dsl] = results[k]["out_l"].reshape(B, C, DLOC, H, W)
        out[:, C:, dsl] = results[k]["out_r"].reshape(B, C, DLOC, H, W)
    rh = out[:, C:]
    for d in range(1, D):
        rh[:, :, d, :, W - d:] = 0
    return out


def _host_inputs_wide(left, right):
    """Per-core inputs for the wide variant: partition p = image bc =
    PBC*k + p, flattened h-major (8192 f32) with a 47-zero tail on the
    right buffer (read only into to-be-zeroed output positions)."""
    lv = left.reshape(B * C, HW)
    rv = right.reshape(B * C, HW)
    in_maps = []
    for k in range(NCORES):
        sl = slice(PBC * k, PBC * (k + 1))
        re = np.zeros((PBC, WSRC), np.float32)
        re[:, :HW] = rv[sl]
        in_maps.append({"left": np.ascontiguousarray(lv[sl]), "rext": re})
    return in_maps


def _gather_wide(results):
    out = np.empty((B, 2 * C, D, H, W), np.float32)
    for k in range(NCORES):
        b0, c0 = divmod(PBC * k, C)
        out[b0, c0:c0 + PBC] = results[k]["out_l"].reshape(PBC, D, H, W)
        out[b0, C + c0:C + c0 + PBC] = results[k]["out_r"].reshape(
            PBC, D, H, W)
    # zero the junk the row-bleeding windows wrote: exactly w + d >= W
    rh = out[:, C:]
    for d in range(1, D):
        rh[:, :, d, :, W - d:] = 0
    return out


def _build_hpipe(repeat):
    """HWDGE-only chunked pipeline: all column-quarter loads stream on the
    SP ring; all quarter stores (left and right interleaved) on the ACT
    ring, each gated only on the quarters it reads. Tests whether HBM
    overlaps reads with writes (win ~15-20us) at no gpsimd cost."""
    Q = 4
    LQ = HW // Q            # 2048 left cols per quarter
    RQ = SLAB // Q          # 2128 right cols per quarter (4*2128+5 = SRCW)
    nc = bass.Bass()
    left_in = nc.declare_dram_parameter(
        "left", [B * C, HW], mybir.dt.float32, isOutput=False)
    rext_in = nc.declare_dram_parameter(
        "rext", [B * C, SRCW], mybir.dt.float32, isOutput=False)
    out_l = nc.declare_dram_parameter(
        "out_l", [B * C, DLOC, HW], mybir.dt.float32, isOutput=True)
    out_r = nc.declare_dram_parameter(
        "out_r", [B * C, DLOC, SLAB], mybir.dt.float32, isOutput=True)

    with (
        nc.sbuf_tensor([B * C, HW], mybir.dt.float32) as left_t,
        nc.sbuf_tensor([B * C, SRCW], mybir.dt.float32) as rext_t,
        nc.semaphore("sst") as sst,
        nc.Block() as block,
    ):
        sld_l = [nc.alloc_semaphore(f"sld_l{q}") for q in range(Q)]
        sld_r = [nc.alloc_semaphore(f"sld_r{q}") for q in range(Q)]
        per_st = 2 * Q * 16

        @block.sync
        def _(sync):
            for r in range(repeat):
                if r:
                    sync.wait_ge(sst, per_st * r)
                for q in range(Q):
                    sync.dma_start(
                        left_t[:, q * LQ:(q + 1) * LQ],
                        left_in[:, q * LQ:(q + 1) * LQ],
                    ).then_inc(sld_l[q], 16)
                    w = RQ if q < Q - 1 else RQ + DLOC - 1
                    sync.dma_start(
                        rext_t[:, q * RQ:q * RQ + w],
                        rext_in[:, q * RQ:q * RQ + w],
                    ).then_inc(sld_r[q], 16)

        @block.scalar
        def _(scalar):
            for r in range(repeat):
                for q in range(Q):
                    scalar.wait_ge(sld_l[q], 16 * (r + 1))
                    lbase = left_t[:]
                    src = type(lbase)(
                        lbase.tensor, lbase.offset + q * LQ,
                        [list(lbase.ap[0]), [0, DLOC], [1, LQ]],
                    )
                    dst = type(out_l[:])(
                        out_l[:].tensor, q * LQ,
                        [[DLOC * HW, B * C], [HW, DLOC], [1, LQ]],
                    )
                    scalar.dma_start(dst, src).then_inc(sst, 16)

                    scalar.wait_ge(sld_r[q], 16 * (r + 1))
                    if q < Q - 1:
                        scalar.wait_ge(sld_r[q + 1], 16 * (r + 1))
                    rbase = rext_t[:]
                    rsrc = type(rbase)(
                        rbase.tensor, rbase.offset + q * RQ,
                        [list(rbase.ap[0]), [1, DLOC], [1, RQ]],
                    )
                    rdst = type(out_r[:])(
                        out_r[:].tensor, q * RQ,
                        [[DLOC * SLAB, B * C], [SLAB, DLOC], [1, RQ]],
                    )
                    scalar.dma_start(rdst, rsrc).then_inc(sst, 16)
            scalar.wait_ge(sst, per_st * repeat)

    return nc


def _build_pipe(nc, repeat, left_in, rext_in, out_l, out_r):
    """Chunked load->store pipeline: loads stream on the gpsimd (SWDGE)
    ring in column quarters; each half's store chain consumes quarters as
    they land, so writes overlap the tail of the reads."""
    Q = 4
    LQ = HW // Q            # 2048 left cols per quarter
    RQ = SLAB // Q          # 2128 right cols per quarter (4*2128+5 = SRCW)
    with (
        nc.sbuf_tensor([B * C, HW], mybir.dt.float32) as left_t,
        nc.sbuf_tensor([B * C, SRCW], mybir.dt.float32) as rext_t,
        nc.semaphore("sst_l") as sst_l,
        nc.semaphore("sst_r") as sst_r,
        nc.Block() as block,
    ):
        # one sem per load quarter: a single DMA inc per iteration, and
        # consumers wait on the full value — intermediate thresholds on a
        # multi-inc sem are racy (per-engine slice completion interleaves).
        sld_l = [nc.alloc_semaphore(f"sld_l{q}") for q in range(Q)]
        sld_r = [nc.alloc_semaphore(f"sld_r{q}") for q in range(Q)]
        per_st = Q * 16

        @block.gpsimd
        def _(gpsimd):
            for r in range(repeat):
                if r:
                    gpsimd.wait_ge(sst_l, per_st * r)
                    gpsimd.wait_ge(sst_r, per_st * r)
                for q in range(Q):
                    gpsimd.dma_start(
                        left_t[:, q * LQ:(q + 1) * LQ],
                        left_in[:, q * LQ:(q + 1) * LQ],
                    ).then_inc(sld_l[q], 16)
                    # right quarter includes the +DLOC-1 tail on the last one
                    w = RQ if q < Q - 1 else RQ + DLOC - 1
                    gpsimd.dma_start(
                        rext_t[:, q * RQ:q * RQ + w],
                        rext_in[:, q * RQ:q * RQ + w],
                    ).then_inc(sld_r[q], 16)

        @block.sync
        def _(sync):
            for r in range(repeat):
                for q in range(Q):
                    sync.wait_ge(sld_l[q], 16 * (r + 1))
                    lbase = left_t[:]
                    src = type(lbase)(
                        lbase.tensor, lbase.offset + q * LQ,
                        [list(lbase.ap[0]), [0, DLOC], [1, LQ]],
                    )
                    dst = type(out_l[:])(
                        out_l[:].tensor, q * LQ,
                        [[DLOC * HW, B * C], [HW, DLOC], [1, LQ]],
                    )
                    sync.dma_start(dst, src).then_inc(sst_l, 16)
            sync.wait_ge(sst_l, per_st * repeat)

        @block.scalar
        def _(scalar):
            for r in range(repeat):
                for q in range(Q):
                    # store quarter q reads src cols [ld+q*RQ, ld+q*RQ+RQ);
                    # ld<DLOC spills DLOC-1 cols into quarter q+1, so wait
                    # for that quarter too (the last quarter's spill is
                    # covered by the widened final load).
                    scalar.wait_ge(sld_r[q], 16 * (r + 1))
                    if q < Q - 1:
                        scalar.wait_ge(sld_r[q + 1], 16 * (r + 1))
                    rbase = rext_t[:]
                    src = type(rbase)(
                        rbase.tensor, rbase.offset + q * RQ,
                        [list(rbase.ap[0]), [1, DLOC], [1, RQ]],
                    )
                    dst = type(out_r[:])(
                        out_r[:].tensor, q * RQ,
                        [[DLOC * SLAB, B * C], [SLAB, DLOC], [1, RQ]],
                    )
                    scalar.dma_start(dst, src).then_inc(sst_r, 16)
            scalar.wait_ge(sst_r, per_st * repeat)

    return nc


def _host_inputs(left, right):
    """Per-core device input dicts (host-side shard prep)."""
    le_flat = np.ascontiguousarray(left.reshape(B * C, HW))
    rf = right.reshape(B * C, H, W)

    in_maps = []
    for k in range(NCORES):
        d0 = DLOC * k
        re = np.zeros((B * C, H, PW), np.float32)
        take = max(0, W - d0)
        re[:, :, :take] = rf[:, :, d0:d0 + take]
        re_flat = np.zeros((B * C, SRCW), np.float32)
        re_flat[:, :SLAB] = re.reshape(B * C, SLAB)
        in_maps.append({"left": le_flat, "rext": re_flat})
    return in_maps


GRADED_VARIANT = "mega2"


def _run(in_maps, variant=None, **kwargs):
    nc = _build(1, variant or GRADED_VARIANT)
    return run_bass_kernel_spmd(nc, in_maps, list(range(NCORES)), **kwargs)


def _gather(results):
    out = np.empty((B, 2 * C, D, H, W), np.float32)
    for k in range(NCORES):
        dsl = slice(DLOC * k, DLOC * (k + 1))
        out[:, :C, dsl] = results[k]["out_l"].reshape(B, C, DLOC, H, W)
        slab_r = results[k]["out_r"].reshape(B, C, DLOC, H, PW)
        out[:, C:, dsl] = slab_r[:, :, :, :, :W]
    return out


def kernel(left_features, right_features, max_disparity):
    left = np.asarray(left_features, dtype=np.float32)
    right = np.asarray(right_features, dtype=np.float32)
    assert int(np.asarray(max_disparity)) == 4 * D
    assert left.shape == (B, C, H, W) and right.shape == (B, C, H, W)

    if GRADED_VARIANT == "repl2":
        in_maps = _host_inputs_repl(left, right)
        res = _run(in_maps)
        return _gather_repl2(res.results)
    if GRADED_VARIANT == "repl":
        in_maps = _host_inputs_repl(left, right)
        res = _run(in_maps)
        return _gather_repl(res.results)
    if GRADED_VARIANT in ("mega2", "mega3", "mega5"):
        in_maps = _host_inputs_mega2(left, right)
        res = _run(in_maps)
        return _gather_mega2(res.results)
    if GRADED_VARIANT in ("wide", "wind", "wind2", "wind4"):
        in_maps = _host_inputs_wide(left, right)
        res = _run(in_maps)
        return _gather_wide(res.results)
    if GRADED_VARIANT == "pshard":
        in_maps = _host_inputs_pshard(left, right)
        res = _run(in_maps)
        return _gather_pshard(res.results)
    if GRADED_VARIANT == "cshard":
        in_maps = _host_inputs_cshard(left, right)
        res = _run(in_maps)
        return _gather_cshard(res.results)
    in_maps = _host_inputs(left, right)
    res = _run(in_maps)
    return _gather(res.results)

